# revision 27
# baseline (speedup 1.0000x reference)
"""Trainium2 Bass kernel for nn_CrossAttention (gnn_message_passing).

Reference computation (per batch b, point n):
  nb[c,n,o]  = sum_f neighbors[c,n,f] * W_two[o,f] + b_two[o]
  q[n,e]     = sum_c pcd[n,c] Wq[e,c]
  scores     = sum_c qc[h,n,c] nb[c,n,o],  qc = (q Wk)/sqrt(8) folded on host
  attn       = softmax_o(scores)
  v[e,n,o]   = sum_c Wv[e,c] nb[c,n,o]
  out[(h,d),n] = sum_o attn[h,n,o] v[(h,d),n,o]

Sharding: data-parallel over (b, n-block): 8 cores, each takes 256 points.

Device pipeline per core (n=256 points in 32 groups of 8, c=64, f=512,
o=256, h=8, d=8). All inputs bf16.

  S1  nb[(j,c), o] via bf16 matmuls: lhsT = host-transposed neighbor
      chunks [f=128, cn=128] (cn = j*64+c, j = point-in-group), rhs =
      W_two^T chunks [f=128, o=256]; 16 matmuls into PSUM [128, 1024].
      Evac fp32->bf16 split across DVE/ACT.
  S2  pair-packed: stationary = nb for a point PAIR [128 = (c of even pt
      | c of odd pt), o-half 128]; rhs = r2 [128, 144] with block
      structure [WvT | qc_even | 0 | 0 ; 0 | 0 | WvT | qc_odd] so one
      K=128 matmul yields v_T and scores_T for BOTH points of the pair:
      out[o-half, 0:64]=v_even, [.,64:72]=scores_even, [.,72:136]=v_odd,
      [.,136:144]=scores_odd. 8 matmuls/group (4 pairs x 2 o-halves).
      r2 is assembled on-chip: static Wv/zero frame (DMA'd once into two
      ping-pong slots) + per-group qc injection (2 small DVE copies).
  exp on ACT directly from PSUM (no max-subtract: |scores| ~ 0.05).
  S4  quad-packed, exp-stationary: stationary = exp for 4 points
      [o-half=128, 32] (cheap 32-col LDWEIGHTS), rhs = v_sb for those 4
      points [o-half, 4 x 72] (cols 64:72 of each 72-block are ones ->
      Z entries). out[8*i+h, 72*i + (e|Z)] per quad; quad B col-tiled to
      partitions 32:64 via tile_position=(0,32). Off-diagonal blocks are
      garbage (mixed o-spaces) and simply never read by the host.
  xc ships per group straight from PSUM; host extracts the diagonal
  blocks and divides by Z (exact math; device does everything else).
"""

import math
import ml_dtypes
import numpy as np
from contextlib import ExitStack

import concourse.bass as bass
import concourse.tile as tile
from concourse import bacc, mybir
from concourse.bass_utils import run_bass_kernel_spmd

F32 = mybir.dt.float32
BF16 = mybir.dt.bfloat16

NCORES = 8
B, N, C, LF = 2, 1024, 64, 256
F2 = 2 * LF          # 512 neighbor features
O = LF               # 256 attention keys per point
H, D = 8, 8          # heads, depth
NP = (B * N) // NCORES  # 256 points per core
G = NP // 8          # 32 groups of 8 points

_BUILD_CACHE = {}


def build_nc(with_bias: bool, repeat: int = 1, g_mod: int = G):
    """Build the per-core Bass module.

    g_mod: number of groups present in the nbt input (the g-loop reads
    nbt[g % g_mod]); g_mod == G for real runs, smaller for timing builds.
    repeat: device-side repetition count (For_i) for timing.
    """
    key = (with_bias, repeat, g_mod)
    if key in _BUILD_CACHE:
        return _BUILD_CACHE[key]

    nc = bacc.Bacc("TRN2", target_bir_lowering=False, debug=False)
    # DRAM I/O
    nbt_d = nc.dram_tensor("nbt", [g_mod, 4, 128, 512], BF16, kind="ExternalInput").ap()
    w2t_d = nc.dram_tensor("w2t", [4, 128, 256], BF16, kind="ExternalInput").ap()
    r2i_d = nc.dram_tensor("r2i", [128, 4, 144], BF16, kind="ExternalInput").ap()
    qcs_d = nc.dram_tensor("qcs", [128, G, 4, 8], BF16, kind="ExternalInput").ap()
    b2_d = nc.dram_tensor("b2", [1, 256], F32, kind="ExternalInput").ap()
    xcout_d = nc.dram_tensor("xcout", [64, G, 260], BF16, kind="ExternalOutput").ap()

    with tile.TileContext(nc) as tc, ExitStack() as ctx:
        singles = ctx.enter_context(tc.tile_pool(name="singles", bufs=1))
        slabs = ctx.enter_context(tc.tile_pool(name="slabs", bufs=3))
        nbs = ctx.enter_context(tc.tile_pool(name="nbs", bufs=2))
        vs = ctx.enter_context(tc.tile_pool(name="vs", bufs=2))
        exps = ctx.enter_context(tc.tile_pool(name="exps", bufs=2))
        xcs = ctx.enter_context(tc.tile_pool(name="xcs", bufs=2))
        ps_nba = ctx.enter_context(tc.tile_pool(name="ps_nba", bufs=1, space="PSUM"))
        ps_nbb = ctx.enter_context(tc.tile_pool(name="ps_nbb", bufs=1, space="PSUM"))
        ps_vq = ctx.enter_context(tc.tile_pool(name="ps_vq", bufs=4, space="PSUM"))
        ps_xc = ctx.enter_context(tc.tile_pool(name="ps_xc", bufs=2, space="PSUM"))

        # one-time loads; only w2t is needed by S1(0) -- the rest (qcs, r2,
        # used first by S2(0)) are DMA'd after slab(0) to shorten the fill.
        w2t = singles.tile([128, 4, 256], BF16)
        nc.sync.dma_start(out=w2t, in_=w2t_d.rearrange("a p c -> p a c"))
        qcs = singles.tile([128, G, 4, 8], BF16)
        # r2 ping-pong: two slots, each [128, 4 pairs, 144]; Wv/zero frame
        # is static, qc cols rewritten per group.
        r2 = singles.tile([128, 2, 4, 144], BF16)

        def emit_singles_dmas():
            nc.sync.dma_start(out=qcs, in_=qcs_d)
            nc.sync.dma_start(out=r2[:, 0], in_=r2i_d)
            nc.sync.dma_start(out=r2[:, 1], in_=r2i_d)
        if with_bias:
            b2 = singles.tile([1, 256], F32)
            nc.sync.dma_start(out=b2, in_=b2_d)
            ones1 = singles.tile([1, 128], F32)
            nc.vector.memset(ones1, 1.0)

        state = {}

        def phase_s1(g):
            """S1 matmuls + nb evac + qc inject for group g."""
            gi = g % g_mod
            pp = g % 2
            slab = slabs.tile([128, 4, 512], BF16, tag="slab")
            nc.sync.dma_start(out=slab, in_=nbt_d[gi].rearrange("a p c -> p a c"))
            # t-order [2,3,0,1]: the ACT-evacuated half (t=2,3) finishes
            # first so ACT never head-blocks on an unready evac.
            nb_b = ps_nbb.tile([128, 512], F32, tag="nbb")
            nb_a = ps_nba.tile([128, 512], F32, tag="nba")
            for t in (2, 3, 0, 1):
                ps = nb_a if t < 2 else nb_b
                col = 256 * (t % 2)
                for ci in range(4):
                    nc.tensor.matmul(
                        ps[:, col : col + 256],
                        slab[:, ci, 128 * t : 128 * t + 128],
                        w2t[:, ci, :],
                        start=(ci == 0),
                        stop=(ci == 3) and not with_bias,
                    )
                if with_bias:
                    nc.tensor.matmul(
                        ps[:, col : col + 256],
                        ones1,
                        b2,
                        start=False,
                        stop=True,
                    )
            # qc inject on Pool (SBUF->SBUF) so DVE/ACT stay on PSUM evacs
            nc.gpsimd.tensor_copy(r2[0:64, pp, :, 64:72], qcs[0:64, g])
            nc.gpsimd.tensor_copy(r2[64:128, pp, :, 136:144], qcs[64:128, g])
            nb_sb = nbs.tile([128, 4, 256], BF16, tag="nb")
            nc.scalar.copy(nb_sb[:, 2:4, :], nb_b)
            nc.vector.tensor_copy(nb_sb[:, 0:2, :], nb_a)
            state[("nb", g)] = nb_sb

        def phase_s2(g):
            """S2 matmuls + v/exp evacs for group g (nb must be ready)."""
            pp = g % 2
            nb_sb = state.pop(("nb", g))
            # v_sb[:, half, pt, 0:64] = v_T, col 64 = ones (Z col)
            v_sb = vs.tile([128, 2, 8, 65], BF16, tag="v")
            nc.gpsimd.memset(v_sb[:, :, :, 64:65], 1.0)
            exp_sb = exps.tile([128, 2, 8, 8], BF16, tag="exp")
            for half in range(2):
                for sub in range(2):  # pairs {0,1} then {2,3}
                    # 176-f32 slot stride keeps both pair-slots in one bank
                    vq = ps_vq.tile([128, 2, 176], F32, tag="vq")
                    for k in range(2):
                        p = 2 * sub + k  # pair index = nb subtile t
                        nc.tensor.matmul(
                            vq[:, k, 0:144],
                            nb_sb[:, p, 128 * half : 128 * half + 128],
                            r2[:, pp, p, :],
                            start=True,
                            stop=True,
                        )
                    # v evac: cols {0:64, 72:136} of both pair-slots
                    src_v = bass.AP(
                        tensor=vq.tensor,
                        offset=vq.offset,
                        ap=[vq.ap[0], [176, 2], [72, 2], [1, 64]],
                    )
                    dst_v = v_sb[:, half, 4 * sub : 4 * sub + 4, 0:64]
                    if half == 0:
                        nc.vector.tensor_copy(dst_v, src_v)
                    else:
                        nc.scalar.copy(dst_v, src_v)
                    # exp: scores cols {64:72, 136:144} straight from PSUM
                    src_s = bass.AP(
                        tensor=vq.tensor,
                        offset=vq.offset + 64,
                        ap=[vq.ap[0], [176, 2], [72, 2], [1, 8]],
                    )
                    nc.scalar.activation(
                        out=exp_sb[:, half, 4 * sub : 4 * sub + 4, :].rearrange(
                            "p a b -> p (a b)"
                        ),
                        in_=src_s,
                        func=mybir.ActivationFunctionType.Exp,
                        scale=1.0,
                    )
            state[("v", g)] = v_sb
            state[("exp", g)] = exp_sb

        def phase_s4(g):
            """S4 matmuls + xc out for group g (v/exp must be ready)."""
            v_sb = state.pop(("v", g))
            exp_sb = state.pop(("exp", g))
            xc_ps = ps_xc.tile([64, 260], F32, tag="xc")
            for quad in range(2):
                for half in range(2):
                    nc.tensor.matmul(
                        xc_ps[32 * quad : 32 * quad + 32, :],
                        exp_sb[:, half, 4 * quad : 4 * quad + 4, :].rearrange(
                            "p a b -> p (a b)"
                        ),
                        v_sb[:, half, 4 * quad : 4 * quad + 4, :].rearrange(
                            "p a b -> p (a b)"
                        ),
                        start=(half == 0),
                        stop=(half == 1),
                        tile_position=(0, 32 * quad),
                    )
            # ship this group's xc (DVE evac -> overlapped DMA out)
            xc_sb = xcs.tile([64, 260], BF16, tag="xcsb")
            nc.vector.tensor_copy(xc_sb, xc_ps)
            nc.sync.dma_start(out=xcout_d[:, g, :], in_=xc_sb)

        def body(_i=None):
            # software pipeline: PE queue sees S1(g), S2(g-1), S4(g-2) so the
            # nb evac of g runs under the S2/S4 PE window and S2/S4 inputs
            # are a full iteration old.
            for g in range(G + 2):
                if g < G:
                    phase_s1(g)
                if g == 0:
                    emit_singles_dmas()
                if 1 <= g <= G:
                    phase_s2(g - 1)
                if 2 <= g:
                    phase_s4(g - 2)

        if repeat > 1:
            with tc.For_i(0, repeat, 1):
                body()
        else:
            body()

    nc.compile()
    _BUILD_CACHE[key] = nc
    return nc


def host_prep(pcd, neighbors, W_two, b_two, Wq, Wk, Wv):
    """Per-core input maps (host-side layout transforms + q/qc fold)."""
    scale = 1.0 / math.sqrt(D)
    # q[b,n,e] then qc[b,h,n,c] = sum_d q[b,n,(h,d)] Wk[(h,d),c] * scale
    q = np.einsum("bnc,ec->bne", pcd, Wq).astype(np.float32)
    qc = np.einsum("bnhd,hdc->bhnc", q.reshape(B, N, H, D), Wk.reshape(H, D, C))
    qc = (qc * scale).astype(np.float32)

    w2t = np.ascontiguousarray(W_two.T.reshape(4, 128, O)).astype(ml_dtypes.bfloat16)
    b2 = b_two.reshape(1, O).astype(np.float32)
    with_bias = bool(np.any(b_two))

    # static r2 frame: [128, 4 pairs, 144]
    r2i = np.zeros((128, 4, 144), np.float32)
    r2i[0:64, :, 0:64] = np.asarray(Wv).T[:, None, :]
    r2i[64:128, :, 72:136] = np.asarray(Wv).T[:, None, :]
    r2i = r2i.astype(ml_dtypes.bfloat16)

    in_maps = []
    npb = N // (NCORES // B)  # points per core
    for core in range(NCORES):
        b = core // (NCORES // B)
        n0 = (core % (NCORES // B)) * npb
        nb = neighbors[b, :, n0 : n0 + npb, :]          # (c, np, f)
        # nbt[g, ci, fi, cn] with cn = (n within group)*64 + c
        nbt = np.transpose(nb, (2, 1, 0)).reshape(F2, G, 8 * C)   # (f, g, cn)
        nbt = np.transpose(nbt, (1, 0, 2)).reshape(G, 4, 128, 8 * C)
        nbt = np.ascontiguousarray(nbt).astype(ml_dtypes.bfloat16)
        # qcs[0:64, g, p, h] = qc[h, n(g,2p), c]; [64:128, ...] odd point
        qc_core = qc[b, :, n0 : n0 + npb, :]             # (h, np, c)
        qr = qc_core.reshape(H, G, 4, 2, C)              # (h, g, pair, par, c)
        qcs = np.empty((128, G, 4, 8), np.float32)
        qcs[0:64] = np.transpose(qr[:, :, :, 0, :], (3, 1, 2, 0))  # (c,g,p,h)
        qcs[64:128] = np.transpose(qr[:, :, :, 1, :], (3, 1, 2, 0))
        qcs = qcs.astype(ml_dtypes.bfloat16)
        in_maps.append({"nbt": nbt, "w2t": w2t, "r2i": r2i, "qcs": qcs, "b2": b2})
    return in_maps, with_bias


def kernel(pcd, neighbors, W_two, b_two, Wq, Wk, Wv):
    in_maps, with_bias = host_prep(pcd, neighbors, W_two, b_two, Wq, Wk, Wv)
    nc = build_nc(with_bias)
    res = run_bass_kernel_spmd(nc, in_maps, list(range(NCORES)))
    out = np.empty((B, C, N), np.float32)
    npb = N // (NCORES // B)
    ee = np.arange(C)                 # output channel e = (h, d)
    hh = ee // D                      # head of channel e
    for core in range(NCORES):
        b = core // (NCORES // B)
        n0 = (core % (NCORES // B)) * npb
        xc = res.results[core]["xcout"].astype(np.float32)   # [64, G, 260]
        # point pt (0..7) of each group: quad qd = pt//4, i = pt%4
        # row = 32*qd + 8*i + h, col = 65*i + e (x) / 65*i + 64 (Z)
        for pt in range(8):
            qd, i = pt // 4, pt % 4
            rows = 32 * qd + 8 * i + hh            # (C,)
            x = xc[rows, :, 65 * i + ee]           # (C, G)
            z = xc[rows, :, 65 * i + 64]           # (C, G)
            out[b, :, n0 + pt : n0 + npb : 8] = x / z
    return out
